# revision 120
# baseline (speedup 1.0000x reference)
"""Causal self-attention (B=2, S=2048, E=1024, H=16) on 8 Trainium2 cores.

Sharding: core c in 0..7 handles batch b = c//4 and the 4 heads
[4*(c%4), 4*(c%4)+4).  The host pre-transposes x[b] and pre-slices the
QKV weights column-wise / Wo row-wise per core; each core computes its
heads' attention plus its partial output projection, and the host sums
the 4 partials per batch.

Device kernel (per core, everything resident in SBUF, matmul inputs in
fp16 with fp32 PSUM accumulation):
  xT [1024,2048] -> QT,KT [d,s] and V [s,d] projections, emitted as
  per-q-block "waves" interleaved into the attention stream (Wq|Wk are
  host-packed into one dram tensor so the startup loads need fewer
  serialized HWDGE issue slots).
  S^T tiles = matmul(lhsT=KT_blk, rhs=QT_blk): k on partitions, q free.
  exp on ScalarE via a two-window AP that skips both heads' causally
  masked prefixes; triangular mask multiply (on Pool, to keep the
  exp-gated mask from head-blocking DVE) on the diagonal 128-blocks.
  S/exp run up to ten steps ahead of the PV stream (software pipeline
  over the 2-buffer S ring, exp results buffered in a 24-deep et ring)
  so PV never pays the exp latency and ACT stays saturated through the
  exp-heavy late blocks.
  PV is "flipped": out[q,d] accumulated per (head, q-subblock) with
  lhsT=P^T tile, rhs=V (+ones column -> softmax denominator l lands in
  the same PSUM row).  PSUM accumulation groups are 2KB-bank-scoped,
  so the 8 chains of a head-pair share two banks with exactly one
  start/stop per bank, and column slots alternate between head-pairs
  so each bank reopen has a tracked WAR on the prior finalize reads.
  Normalization is a per-partition reciprocal + one bank-wide broadcast
  multiply (l is on partitions in this layout), then a DMA-xbar transpose puts
  O^T [d,q] in SBUF for the output projection; the last head-pair uses
  a PE transpose instead so the final Y chains start without the
  ~2.4us DMA-transpose latency.
  Y = O @ Wo streamed out per s-block as fp16 (host sums partials in
  fp32); Y(qb3) is emitted inline right after its transposes so only
  s-block 15's store tails out.  Projection/Y chains are split into
  ~2-4-matmul units and paced into per-block fill windows to keep PE
  fed while ACT (exp) paces the softmax.
"""

import numpy as np
from contextlib import ExitStack

B, S, E, H, D = 2, 2048, 1024, 16, 64
N_CORES = 8
CPB = 4              # cores per batch
HL = H // CPB        # heads per core = 4
DL = HL * D          # local head dims = 256
P = 128              # partitions
EC = E // P          # 8 e-chunks
SB = S // P          # 16 s/k blocks
NQB = S // 512       # 4 q blocks of 512
MT = DL // P         # 2 row-tiles of QT/KT/OT (2 heads each)

_CACHE = {}
_EXHAUSTED = object()


def _emit(ctx, tc, xT, wqk, wv, wo, mask, y, loop_n=0):
    import concourse.bass as bass  # noqa: F401
    from concourse import mybir

    nc = tc.nc
    f32 = mybir.dt.float32
    f16 = mybir.dt.float16
    Exp = mybir.ActivationFunctionType.Exp
    Copy = mybir.ActivationFunctionType.Copy

    res = ctx.enter_context(tc.tile_pool(name="res", bufs=1))
    xt_sb = res.tile([P, EC, S], f16, tag="xt")
    wqk_sb = res.tile([P, EC, 2 * DL], f16, tag="wqk")
    wq_sb = wqk_sb[:, :, 0:DL]
    wk_sb = wqk_sb[:, :, DL:2 * DL]
    wv_sb = res.tile([P, EC, DL], f16, tag="wv")
    wo_sb = res.tile([P, MT, E], f16, tag="wo")
    qt_sb = res.tile([P, MT, S], f16, tag="qt")
    kt_sb = res.tile([P, MT, S], f16, tag="kt")
    vt_sb = res.tile([P, SB, HL, D + 1], f16, tag="vt")
    ot_sb = res.tile([P, MT, S], f16, tag="ot")
    mi_sb = res.tile([P, 2 * P], f16, tag="mi")
    mask_sb = mi_sb[:, 0:P]
    ident_sb = mi_sb[:, P:2 * P]

    # PSUM: 4 + 2 + 2 = 8 banks exactly.
    s_ps = ctx.enter_context(tc.tile_pool(name="sps", bufs=2, space="PSUM"))
    acc_ps = ctx.enter_context(tc.tile_pool(name="acc", bufs=1, space="PSUM"))
    mm_ps = ctx.enter_context(tc.tile_pool(name="mm", bufs=2, space="PSUM"))

    pre_pool = ctx.enter_context(tc.tile_pool(name="pp", bufs=30))
    oq_pool = ctx.enter_context(tc.tile_pool(name="oq", bufs=10))
    rec_pool = ctx.enter_context(tc.tile_pool(name="rp", bufs=6))
    y_pool = ctx.enter_context(tc.tile_pool(name="yp", bufs=8))

    def _full_body():
        dma = nc.sync

        # ---- loads: small priority pieces first so PE starts ~4us in ----
        def wslice(w, lo, hi):
            return w[lo * P:hi * P, :].rearrange("(ec p) d -> p ec d", p=P)

        def xslice(lo, hi, s0, s1):
            return xT[lo * P:hi * P, s0:s1].rearrange("(ec p) s -> p ec s", p=P)

        for lo, hi in ((0, 1), (1, 4), (4, EC)):  # (wqk, x) duos
            dma.dma_start(out=wqk_sb[:, lo:hi, :], in_=wslice(wqk, lo, hi))
            dma.dma_start(out=xt_sb[:, lo:hi, 0:512], in_=xslice(lo, hi, 0, 512))
        dma.dma_start(out=wv_sb[:], in_=wslice(wv, 0, EC))
        dma.dma_start(out=xt_sb[:, :, 512:1024], in_=xslice(0, EC, 512, 1024))
        dma.dma_start(out=xt_sb[:, :, 1024:1536], in_=xslice(0, EC, 1024, 1536))
        dma.dma_start(out=xt_sb[:, :, 1536:2048], in_=xslice(0, EC, 1536, 2048))
        dma.dma_start(
            out=wo_sb[:], in_=wo.rearrange("(mt p) e -> p mt e", p=P))
        dma.dma_start(out=mi_sb[:], in_=mask[:])
        nc.vector.memset(vt_sb[:, :, :, D:D + 1], 1.0)

        # ---- wave 0: DMA-paced; 4 parallel qt/kt chains (2 psum tiles
        # borrowed from the idle s pool), then V chains ----
        pq0 = mm_ps.tile([P, 512], f32, tag="mm")
        pq1 = mm_ps.tile([P, 512], f32, tag="mm")
        spb = s_ps.tile([P, 1024], f32, tag="s")
        w0 = [(wq_sb, qt_sb, 0, pq0[:]), (wq_sb, qt_sb, 1, pq1[:]),
              (wk_sb, kt_sb, 0, spb[:, 0:512]), (wk_sb, kt_sb, 1, spb[:, 512:1024])]
        for ec in range(EC):
            for w_sb, t_sb, mt, pch in w0:
                nc.tensor.matmul(
                    pch, w_sb[:, ec, mt * P:(mt + 1) * P],
                    xt_sb[:, ec, 0:512],
                    start=(ec == 0), stop=(ec == EC - 1))
        for w_sb, t_sb, mt, pch in w0:
            nc.vector.tensor_copy(t_sb[:, mt, 0:512], pch)
        for sb in range(4):
            ps = mm_ps.tile([P, 512], f32, tag="mm")
            for ec in range(EC):
                nc.tensor.matmul(
                    ps[:, 0:DL],
                    xt_sb[:, ec, sb * P:(sb + 1) * P],
                    wv_sb[:, ec, :],
                    start=(ec == 0), stop=(ec == EC - 1))
            nc.vector.tensor_copy(
                vt_sb[:, sb, :, 0:D],
                ps[:, 0:DL].rearrange("p (h d) -> p h d", h=HL))

        def wave_units(nb, parts=("qt", "kt", "v"), sbs=None):
            # QT/KT [:, :, nb-window] = (w chunk)^T @ xT ; V[4nb..4nb+3].
            srcs = []
            if "qt" in parts:
                srcs.append((wq_sb, qt_sb))
            if "kt" in parts:
                srcs.append((wk_sb, kt_sb))
            for mt in range(MT):
                for w_sb, t_sb in srcs:
                    ps = mm_ps.tile([P, 512], f32, tag="mm")
                    for ec in range(EC):
                        nc.tensor.matmul(
                            ps[:],
                            w_sb[:, ec, mt * P:(mt + 1) * P],
                            xt_sb[:, ec, nb * 512:(nb + 1) * 512],
                            start=(ec == 0), stop=(ec == EC - 1))
                        if ec in (1, 3, 5):
                            yield
                    nc.vector.tensor_copy(
                        t_sb[:, mt, nb * 512:(nb + 1) * 512], ps[:])
                    yield
            if "v" not in parts:
                return
            for sb in (sbs if sbs is not None
                       else range(4 * nb, 4 * nb + 4)):
                ps = mm_ps.tile([P, 512], f32, tag="mm")
                for ec in range(EC):
                    nc.tensor.matmul(
                        ps[:, 0:DL],
                        xt_sb[:, ec, sb * P:(sb + 1) * P],
                        wv_sb[:, ec, :],
                        start=(ec == 0), stop=(ec == EC - 1))
                    if ec in (1, 3, 5):
                        yield
                nc.vector.tensor_copy(
                    vt_sb[:, sb, :, 0:D],
                    ps[:, 0:DL].rearrange("p (h d) -> p h d", h=HL))
                yield

        def out_proj_sb(sb, tail=False, act_copy=False):
            # Y[sb, :] = O[sb, :] @ wo, staged in fp16 and stored per s-block.
            # The tail variant splits copy engines + stores so the kernel's
            # final store chain is as short as possible.
            yt = y_pool.tile([P, E], f16, tag="y")
            for eb in range(E // 512):
                yp = mm_ps.tile([P, 512], f32, tag="mm")
                for dc in range(MT):
                    nc.tensor.matmul(
                        yp[:],
                        ot_sb[:, dc, sb * P:(sb + 1) * P],
                        wo_sb[:, dc, eb * 512:(eb + 1) * 512],
                        start=(dc == 0), stop=(dc == MT - 1))
                # DVE, not Pool: GPSIMD cannot access PSUM on real HW.
                # Inline Y(qb3) units run after ACT's exp stream drains, and
                # Copy shares exp's act table, so their eb0 copies ride
                # ScalarE off the DVE-congested tail.
                if act_copy and eb == 0:
                    nc.scalar.activation(out=yt[:, 0:512], in_=yp[:],
                                         func=Copy, scale=1.0)
                else:
                    nc.vector.tensor_copy(
                        yt[:, eb * 512:(eb + 1) * 512], yp[:])
                yield
            dma.dma_start(out=y[sb * P:(sb + 1) * P, :], in_=yt[:])

        def out_proj_units(qb):
            for sb in range(4 * qb, 4 * qb + 4):
                yield from out_proj_sb(sb)

        scale = float(1.0 / np.sqrt(D))
        warm = {}   # (qb, mt, kb) -> et, produced across block boundaries

        def emit_se_abs(qb_, mt_, kb_):
            t_ = kb_ - 4 * qb_
            v0_ = P * t_ if t_ > 0 else 0
            sp = s_ps.tile([P, 1024], f32, tag="s")
            for half in range(2):
                dr = half * D
                nc.tensor.matmul(
                    sp[:, half * 512 + v0_:(half + 1) * 512],
                    kt_sb[dr:dr + D, mt_, kb_ * P:(kb_ + 1) * P],
                    qt_sb[dr:dr + D, mt_, qb_ * 512 + v0_:(qb_ + 1) * 512],
                    start=True, stop=True)
            et = pre_pool.tile([P, 1024], f16, tag="pe")
            nc.scalar.activation(
                out=et.rearrange("p (h q) -> p h q", h=2)[:, :, v0_:512],
                in_=sp.rearrange("p (h q) -> p h q", h=2)[:, :, v0_:512],
                func=Exp, scale=scale)
            if t_ >= 0:
                eng = nc.vector if (qb_ == NQB - 1 and mt_ == 1) \
                    else nc.gpsimd
                for half in range(2):
                    w0 = half * 512 + v0_
                    eng.tensor_mul(
                        et[:, w0:w0 + P], et[:, w0:w0 + P], mask_sb[:])
            warm[(qb_, mt_, kb_)] = et

        gstep = [0]   # global consume-step counter across blocks

        def attention_block(qb, fills, warmup=()):
            # Consume side: flipped PV accumulates O[q,d]+l per (head,
            # q-subblock) in one 2-bank PSUM tile reused across mt; S/exp
            # come from the produce stream's et buffer.  fills: list of
            # [gen, n_units, frac, ready_flag] paced independently so
            # deadline-bound work (next wave's KT/V) front-loads while Y
            # chains spread over the whole block.
            nkb = 4 * (qb + 1)     # causal: k blocks 0 .. nkb-1
            nsteps = MT * nkb
            # fills entries: [gen, n_units, s0, s1] — units paced linearly
            # over the step window [s0*nsteps, s1*nsteps).
            state = []
            for f in fills:
                g, n = f[0], f[1]
                s0, s1 = (0.0, f[2]) if len(f) == 3 else (f[2], f[3])
                state.append([g, n, s0 * nsteps, max(1e-9, (s1 - s0) * nsteps),
                              0])

            def run_fill(step):
                for st in state:
                    g, n, lo, width, done = st
                    want = min(n, max(0, int((step + 1 - lo) * n / width)))
                    while st[4] < want:
                        if next(g, _EXHAUSTED) is _EXHAUSTED:
                            st[4] = n
                            break
                        st[4] += 1

            # For the last q-block, Y(sb) is emitted inline shortly after
            # its (mt=1, qs) transpose lands so only sb15's Y tails out.
            deferred = []   # (legal_step, generator)

            def run_deferred(step):
                for i in range(len(deferred) - 1, -1, -1):
                    legal, g = deferred[i]
                    if step >= legal:
                        for _ in g:
                            pass
                        deferred.pop(i)

            # Software pipeline: S/exp/mask run up to 10 steps ahead of
            # the PV stream (et ring buffers the results), so PV never
            # waits out the ~1.3us exp latency and ACT works ahead through
            # the exp-heavy late blocks.
            steps = [(mt_, kb_) for mt_ in range(MT) for kb_ in range(nkb)]
            emitted = {}
            nprod = [0]

            warm_next = list(warmup)

            def emit_se(i):
                mt_, kb_ = steps[i]
                if (qb, mt_, kb_) in warm:
                    emitted[i] = warm.pop((qb, mt_, kb_))
                    return
                t_ = kb_ - 4 * qb
                v0_ = P * t_ if t_ > 0 else 0
                sp = s_ps.tile([P, 1024], f32, tag="s")
                for half in range(2):
                    dr = half * D
                    nc.tensor.matmul(
                        sp[:, half * 512 + v0_:(half + 1) * 512],
                        kt_sb[dr:dr + D, mt_, kb_ * P:(kb_ + 1) * P],
                        qt_sb[dr:dr + D, mt_,
                              qb * 512 + v0_:(qb + 1) * 512],
                        start=True, stop=True)
                et = pre_pool.tile([P, 1024], f16, tag="pe")
                nc.scalar.activation(
                    out=et.rearrange("p (h q) -> p h q", h=2)[:, :, v0_:512],
                    in_=sp.rearrange("p (h q) -> p h q", h=2)[:, :, v0_:512],
                    func=Exp, scale=scale)
                if t_ >= 0:  # diagonal block: mask future keys.
                    # On Pool (SBUF-only op): keeps the exp-gated mask from
                    # head-blocking DVE's finalize stream.  The last
                    # head-pair's masks go to DVE instead: its Y-copy load
                    # moved to ACT, and Pool's q7-launch overhead was
                    # stalling the final diagonal PVs.
                    eng = nc.vector if (qb == NQB - 1 and mt_ == 1) \
                        else nc.gpsimd
                    for half in range(2):
                        w0 = half * 512 + v0_
                        eng.tensor_mul(
                            et[:, w0:w0 + P], et[:, w0:w0 + P], mask_sb[:])
                emitted[i] = et

            step = 0
            acc = None
            for mt, kb in steps:
                t = kb - 4 * qb
                v0 = P * t if t > 0 else 0   # masked prefix of window
                if kb == 0:
                    acc = acc_ps.tile([P, 1024], f32, tag="a")
                    # PSUM accumulation groups are 2KB-bank-scoped: one
                    # start/stop per bank per head-pair; qs0/qs1 chains in
                    # bank 0, qs2/qs3 in bank 1.  Column slots alternate
                    # between head-pairs so each bank's opening matmul has
                    # a tracked WAR on the prior pair's finalize reads.
                par = (qb * MT + mt) % 2

                def col0(qs, par=par):
                    return (qs // 2) * 512 + ((qs + par) % 2) * 130
                # ramp the lookahead (3/step) instead of bursting all 10
                # at block start: a burst head-blocks PE on the 2-deep S
                # ring behind ACT's exp queue
                target = min(len(steps), step + 11, 5 * (step + 1))
                while nprod[0] < target:
                    emit_se(nprod[0])
                    nprod[0] += 1
                if warm_next and step >= len(steps) - len(warmup):
                    emit_se_abs(*warm_next.pop(0))
                et = emitted.pop(step)
                run_fill(step)
                step += 1
                gstep[0] += 1
                if True:
                    for half in range(2):
                        h = 2 * mt + half
                        for qs in range(max(t, 0), 4):
                            c0 = col0(qs) + half * 65
                            start = kb == 0 and half == 0 and qs % 2 == 0
                            stop = half == 1 and kb == 4 * qb + qs \
                                and qs % 2 == 1
                            nc.tensor.matmul(
                                acc[:, c0:c0 + 65],
                                et[:, half * 512 + qs * P:half * 512 + (qs + 1) * P],
                                vt_sb[:, kb, h, :],
                                start=start, stop=stop)
                    if t == 1 or t == 3:
                        # The bank holding q-subblocks (t-1, t) just closed
                        # its accumulation group (reads of an open bank are
                        # illegal): normalize both (l is per-partition) and
                        # transpose out.
                        b0 = (t // 2) * 512
                        rec = rec_pool.tile([P, 4], f32, tag="r")
                        nc.vector.reciprocal(
                            rec[:], acc[:, b0 + 64:b0 + 260:65])
                        last = qb == NQB - 1 and mt == MT - 1
                        # one broadcast multiply normalizes the whole bank
                        # (4 chains x 64), instead of four tensor_scalars
                        oq2 = oq_pool.tile([P, 2 * P], f16, tag="o")
                        nc.vector.tensor_mul(
                            oq2.rearrange("p (a b) -> p a b", b=64),
                            acc[:, b0:b0 + 260].rearrange(
                                "p (a b) -> p a b", b=65)[:, :, 0:64],
                            rec.broadcast_to((P, 4, 64)))
                        for qs in (t - 1, t):
                            sl = (qs + par) % 2
                            oq = oq2[:, sl * P:(sl + 1) * P]
                            owin = ot_sb[:, mt, qb * 512 + qs * P:
                                         qb * 512 + (qs + 1) * P]
                            if last:
                                # last head-pair: PE xbar transpose (+DVE
                                # copy) beats the ~2.4us DMA-transpose fixed
                                # latency, so Y(sb) can follow immediately
                                tr = s_ps.tile([P, P], f16, tag="s")
                                nc.tensor.transpose(tr[:], oq[:], ident_sb[:])
                                nc.vector.tensor_copy(owin, tr[:])
                                if qs == 3:
                                    run_deferred(step + 1000)  # drain rest
                                deferred.append(
                                    (step + 1, out_proj_sb(4 * qb + qs,
                                                           tail=(qs == 3),
                                                           act_copy=True)))
                            else:
                                dma.dma_start_transpose(out=owin, in_=oq[:])
                    run_deferred(step)
            # drain leftover fill, then any remaining deferred Y units
            for st in state:
                while next(st[0], _EXHAUSTED) is not _EXHAUSTED:
                    pass
            run_deferred(10 ** 6)

        # Fill plan: waves run as fill in earlier attention blocks (late
        # phases are ACT-bound, so Y work is pushed there); V(3) is
        # front-loaded inside attn(3) to land before its kb=12 diagonal;
        # Y(3) is emitted inline at attn(3)'s last diagonal steps.
        def _chain(*gs):
            for g in gs:
                yield from g

        plans = [
            [[wave_units(1), 32, 0.95]],
            [[wave_units(2), 32, 1.0]],
            [[wave_units(3, parts=("qt", "kt")), 16, 0.8],
             [out_proj_units(0), 8, 1.0]],
            [[wave_units(3, parts=("v",)), 16, 0.5],
             [out_proj_units(1), 8, 0.05, 0.5],
             [out_proj_units(2), 8, 0.2, 0.7]],
        ]
        for qb, fills in enumerate(plans):
            # cap warmup so its qt/kt reads are emitted only after the
            # corresponding wave fill units (attn(0) is only 8 steps long)
            nxt = [(qb + 1, 0, kb)
                   for kb in range(3 if qb == 0 else 12)] \
                if qb < NQB - 1 else ()
            attention_block(qb, fills, warmup=nxt)

    if loop_n:
        # bench-only path: hint all engines so the back-edge prefetches
        # the body's IRAM blocks (body >256 instructions per engine)
        hints = (mybir.EngineType.PE, mybir.EngineType.Activation,
                 mybir.EngineType.DVE, mybir.EngineType.SP,
                 mybir.EngineType.Pool)
        with tc.For_i(0, loop_n, 1, hint_engines=hints):
            _full_body()
    else:
        _full_body()


def _get_program(loop_n=0):
    key = ("nc", loop_n)
    if key in _CACHE:
        return _CACHE[key]
    import concourse.tile as tile
    from concourse import bacc, mybir

    f16 = mybir.dt.float16
    nc = bacc.Bacc("TRN2", target_bir_lowering=False, debug=False,
                   enable_asserts=False)
    xT = nc.dram_tensor("xT", [E, S], f16, kind="ExternalInput").ap()
    wqk = nc.dram_tensor("wqk", [E, 2 * DL], f16, kind="ExternalInput").ap()
    wv = nc.dram_tensor("wv", [E, DL], f16, kind="ExternalInput").ap()
    wo = nc.dram_tensor("wo", [DL, E], f16, kind="ExternalInput").ap()
    mask = nc.dram_tensor("mask", [P, 2 * P], f16, kind="ExternalInput").ap()
    y = nc.dram_tensor("y", [S, E], f16, kind="ExternalOutput").ap()
    with tile.TileContext(nc) as tc:
        with ExitStack() as ctx:
            _emit(ctx, tc, xT, wqk, wv, wo, mask, y, loop_n=loop_n)
    nc.compile()
    _CACHE[key] = nc
    return nc


def _make_in_maps(x, Wq, Wk, Wv, Wo):
    x = np.asarray(x, dtype=np.float32)
    Wq = np.asarray(Wq, dtype=np.float32)
    Wk = np.asarray(Wk, dtype=np.float32)
    Wv = np.asarray(Wv, dtype=np.float32)
    Wo = np.asarray(Wo, dtype=np.float32)
    mi = np.concatenate([np.triu(np.ones((P, P), np.float16)),
                         np.eye(P, dtype=np.float16)], axis=1)
    in_maps = []
    for c in range(N_CORES):
        b, hg = divmod(c, CPB)
        hs = slice(hg * HL, (hg + 1) * HL)
        wqs = Wq.reshape(E, H, D)[:, hs, :].reshape(E, DL)
        wks = Wk.reshape(E, H, D)[:, hs, :].reshape(E, DL)
        in_maps.append({
            "xT": np.ascontiguousarray(x[b].T).astype(np.float16),
            "wqk": np.ascontiguousarray(
                np.concatenate([wqs, wks], axis=1)).astype(np.float16),
            "wv": np.ascontiguousarray(Wv.reshape(E, H, D)[:, hs, :].reshape(E, DL)).astype(np.float16),
            "wo": np.ascontiguousarray(Wo.reshape(H, D, E)[hs, :, :].reshape(DL, E)).astype(np.float16),
            "mask": mi,
        })
    return in_maps


def run(x, Wq, Wk, Wv, Wo, trace=False):
    from concourse.bass_utils import run_bass_kernel_spmd

    nc = _get_program()
    in_maps = _make_in_maps(x, Wq, Wk, Wv, Wo)
    br = run_bass_kernel_spmd(nc, in_maps, list(range(N_CORES)), trace=trace)
    out = np.zeros((B, S, E), dtype=np.float32)
    for c in range(N_CORES):
        out[c // CPB] += br.results[c]["y"].astype(np.float32)
    return out, br


def kernel(x, Wq, Wk, Wv, Wo):
    out, _ = run(x, Wq, Wk, Wv, Wo, trace=False)
    return out


if __name__ == "__main__":
    from concourse.timeline_sim import TimelineSim
    nc = _get_program()
    est = TimelineSim(nc, trace=False).simulate()
    print(f"TimelineSim estimate: {est:.0f} ns")
